# revision 12
# baseline (speedup 1.0000x reference)
"""BSCA-unrolled Trainium2 Bass kernel (nn_BSCAUnrolled).

Data-parallel over the batch dim B=8 across 8 NeuronCores; each core runs
the full 5-layer BSCA update for one batch element:

    per layer: err = Om*(Y - R@A); P/Q masked ridge solves (batched 10x10
    Gaussian elimination on DVE); A soft-threshold BSCA step.

Key algebraic transforms vs the reference:
  - A_scale (and inv = 1/A_scale) depend only on R, Omega -> computed once.
  - datafit = err - Om*(P_new@Q_new^T) reuses the R@A GEMM from err.
  - A_scale==0 cannot occur for this data (an all-zero Omega column has
    probability 0.1^512), so A_new = A + sA*(C - clamp(A_scale*A + C,
    -mu, mu))*inv with C = R^T@datafit.

All matmul operands are float32r (~1e-4 matmul rel-err at full PE rate).
The carry A and err live in f32r SBUF; R/R^T/Om^T/A_scale/inv are staged
once to HBM in f32r and streamed per layer; solves run in fp32 on DVE.
"""
import sys
import numpy as np

for _p in ("/opt/trn_rl_repo", "/root/.axon_site/_ro/trn_rl_repo"):
    if _p not in sys.path:
        sys.path.insert(0, _p)

import concourse.bass as bass
import concourse.mybir as mybir
import concourse.tile as tile
from concourse import bacc
from concourse.bass_utils import run_bass_kernel_spmd
from concourse.masks import make_identity

F32 = mybir.dt.float32
F32R = mybir.dt.float32r
BF16 = mybir.dt.bfloat16
AL = mybir.AluOpType
AX = mybir.AxisListType

B, E, F, T, RK, L = 8, 512, 1024, 2048, 10, 5
EC, FC, TC = E // 128, F // 128, T // 128      # 4, 8, 16
N4 = T // 512                                  # 4 column stripes
W_ = RK + 1                                    # augmented row width

PRM_LAM, PRM_MU, PRM_NMU, PRM_SP, PRM_SQ, PRM_SA = range(6)
NPRM = 6 * L

KCFG = {"layers": L, "dbg": False, "use_soft_resid": True}

_SOFT_RESID = 0  # 0 = not attempted, None = failed


def _register_soft_resid():
    """Custom DVE op: out = Src1 - clamp(Src0 + Src1, C0, C1)."""
    global _SOFT_RESID
    if _SOFT_RESID != 0:
        return _SOFT_RESID
    _SOFT_RESID = None
    try:
        import concourse.dve_ops as dve_ops
        from concourse.dve_spec import (
            Spec, Src0, Src1, C0, C1, minn, maxx, lower, _has_src1,
        )
        from concourse.dve_uop import DveOpSpec

        name = "SOFT_RESID_ANT"
        for o in dve_ops.OPS:
            if o.name == name:
                _SOFT_RESID = o
                return o

        def _ref(in0, in1, s0, s1, imm2):
            ba = (in0.astype(np.float32) + in1).astype(np.float32)
            return (in1 - np.clip(ba, s0, s1)).astype(np.float32)

        spec = Spec(body=Src1 - minn(maxx(Src0 + Src1, C0), C1), reference=_ref)
        row = dve_ops._CUSTOM_DVE_ROW_BASE + len(dve_ops.OPS)
        shas = {}
        for ver in ("v3", "v4"):
            try:
                s = DveOpSpec(name=name, opcode=row, uops=lower(spec, ver=ver),
                              rd1_en=_has_src1(spec))
                shas[ver] = s.sha(ver)
            except Exception:
                pass
        if "v3" not in shas:
            return None
        op = dve_ops.DveOp(name, spec, subdim=False, uops_sha=shas)
        dve_ops.OPS.append(op)
        dve_ops.CUSTOM_DVE_SPECS[name] = spec
        dve_ops._SUB_OPCODE_FOR_NAME[name] = row
        _SOFT_RESID = op
        return op
    except Exception:
        return None


def _gauss_solve(nc, pool, ws, s, tag, xtag):
    """Batched solve of s systems/partition, each 10x10 (SPD, no pivoting).

    ws: [128, s, 10, 11] fp32, rows = [M | rhs]. Destroys ws.
    Returns xout [128, s, 10] fp32.
    """
    P = 128
    for k in range(RK - 1):
        nr = RK - 1 - k          # rows below pivot
        nw = W_ - k              # remaining row width (cols k..10)
        rpiv = pool.tile([P, s, 1], F32, tag=f"{tag}_rpiv")
        nc.vector.reciprocal(rpiv[:], ws[:, :, k, k, None])
        cf = pool.tile([P, s, RK - 1], F32, tag=f"{tag}_cf")
        nc.vector.tensor_mul(cf[:, :, :nr], ws[:, :, k + 1:, k],
                             rpiv[:].to_broadcast((P, s, nr)))
        prod = pool.tile([P, s, RK - 1, W_], F32, tag=f"{tag}_prod")
        nc.vector.tensor_mul(
            prod[:, :, :nr, :nw],
            cf[:, :, :nr, None].to_broadcast((P, s, nr, nw)),
            ws[:, :, k, None, k:].to_broadcast((P, s, nr, nw)),
        )
        nc.vector.tensor_sub(ws[:, :, k + 1:, k:], ws[:, :, k + 1:, k:],
                             prod[:, :, :nr, :nw])
    # back substitution
    rdiag = pool.tile([P, s, RK], F32, tag=f"{tag}_rdiag")
    diag_view = ws.rearrange("p s a b -> p s (a b)")[:, :, 0:RK * W_:W_ + 1]
    nc.vector.reciprocal(rdiag[:], diag_view)
    xout = pool.tile([P, s, RK], F32, tag=xtag)
    nc.vector.tensor_mul(xout[:, :, RK - 1, None], ws[:, :, RK - 1, RK, None],
                         rdiag[:, :, RK - 1, None])
    for i in range(RK - 2, -1, -1):
        nt = RK - 1 - i          # tail length
        tl = pool.tile([P, s, RK - 1], F32, tag=f"{tag}_tl")
        nc.vector.tensor_mul(tl[:, :, :nt], ws[:, :, i, i + 1:RK],
                             xout[:, :, i + 1:])
        red = pool.tile([P, s, 1], F32, tag=f"{tag}_red")
        nc.vector.tensor_reduce(red[:], tl[:, :, :nt], AX.X, AL.add)
        nc.vector.tensor_sub(red[:], ws[:, :, i, RK, None], red[:])
        nc.vector.tensor_mul(xout[:, :, i, None], red[:], rdiag[:, :, i, None])
    return xout


def build_bass():
    soft_resid = _register_soft_resid() if KCFG["use_soft_resid"] else None
    n_layers = KCFG["layers"]
    dbg = KCFG["dbg"]
    nc = bacc.Bacc("TRN2", target_bir_lowering=False, debug=False,
                   num_devices=8)

    # ---- external I/O (per core = one batch element) ----
    Y_d = nc.dram_tensor("Y", [E, T], F32, kind="ExternalInput").ap()
    R_d = nc.dram_tensor("R", [E, F], F32, kind="ExternalInput").ap()
    Om_d = nc.dram_tensor("Om", [E, T], F32, kind="ExternalInput").ap()
    P0_d = nc.dram_tensor("P0", [E, RK], F32, kind="ExternalInput").ap()
    Q0_d = nc.dram_tensor("Q0", [T, RK], F32, kind="ExternalInput").ap()
    A0_d = nc.dram_tensor("A0", [F, T], F32, kind="ExternalInput").ap()
    prm_d = nc.dram_tensor("prm", [128, NPRM], F32, kind="ExternalInput").ap()

    Po_d = nc.dram_tensor("Pout", [E, RK], F32, kind="ExternalOutput").ap()
    Qo_d = nc.dram_tensor("Qout", [T, RK], F32, kind="ExternalOutput").ap()
    Ao_d = nc.dram_tensor("Aout", [F, T], F32, kind="ExternalOutput").ap()
    dbg_d = {}
    if dbg:
        for nm, shp in [("err0", [128, EC, T]), ("wsP", [128, EC, RK, W_]),
                        ("phat", [128, EC, RK]), ("wsQ", [128, TC, RK, W_]),
                        ("qhat", [128, TC, RK]), ("df", [128, EC, T]),
                        ("asc", [128, FC, T]), ("inv", [128, FC, T])]:
            dbg_d[nm] = nc.dram_tensor("dbg_" + nm, shp, F32,
                                       kind="ExternalOutput").ap()

    # ---- internal HBM staging for per-layer streamed operands ----
    omr_d = nc.dram_tensor("omr_hbm", [128, EC, T], F32R).ap()
    ombf_d = nc.dram_tensor("ombf_hbm", [128, EC, T], BF16).ap()
    ot_d = nc.dram_tensor("ot_hbm", [128, TC, E], F32).ap()
    inv_d = nc.dram_tensor("inv_hbm", [128, FC, T], F32R).ap()
    asc_d = nc.dram_tensor("asc_hbm", [128, FC, T], F32R).ap()
    rr_d = nc.dram_tensor("rr_hbm", [128, EC, F], F32R).ap()
    rtr_d = nc.dram_tensor("rtr_hbm", [128, FC, E], F32R).ap()

    # DRAM views in sbuf partition layout
    Yv = Y_d.rearrange("(c p) t -> p c t", p=128)       # [128, EC, T]
    Rv = R_d.rearrange("(c p) f -> p c f", p=128)       # [128, EC, F]
    Omv = Om_d.rearrange("(c p) t -> p c t", p=128)
    P0v = P0_d.rearrange("(c p) r -> p c r", p=128)
    Q0v = Q0_d.rearrange("(a p) r -> p a r", p=128)
    A0v = A0_d.rearrange("(k p) t -> p k t", p=128)
    Pov = Po_d.rearrange("(c p) r -> p c r", p=128)
    Qov = Qo_d.rearrange("(a p) r -> p a r", p=128)
    Aov = Ao_d.rearrange("(k p) t -> p k t", p=128)

    with tile.TileContext(nc) as tc:
        with tc.tile_pool(name="resA", bufs=1) as resA, \
             tc.tile_pool(name="pspan", bufs=2, space="PSUM") as pspan, \
             tc.tile_pool(name="psm", bufs=2, space="PSUM") as psm:

            # ================= one-time preprocessing =================
            id_r = resA.tile([128, 128], F32R)
            id_f = resA.tile([128, 128], F32)
            prm = resA.tile([128, NPRM], F32)
            nc.sync.dma_start(prm[:], prm_d[:])

            def pcol(l, which):
                return prm[:, l * 6 + which, None]

            with tc.tile_pool(name="stgR", bufs=1) as stg, \
                 tc.tile_pool(name="stgo", bufs=2) as stgo:
                make_identity(nc, id_f[:])
                nc.vector.tensor_copy(id_r[:], id_f[:])

                # R: load, cast -> HBM, square, transpose -> HBM
                r_stage = stg.tile([128, EC, F], F32)
                nc.sync.dma_start(r_stage[:], Rv[:])
                R_r = stg.tile([128, EC, F], F32R)
                nc.vector.tensor_copy(R_r[:], r_stage[:])
                nc.sync.dma_start(rr_d[:], R_r[:])
                R2_r = stg.tile([128, EC, F], F32R)
                nc.scalar.activation(R2_r[:], r_stage[:],
                                     mybir.ActivationFunctionType.Square)
                for c in range(EC):
                    for k in range(FC):
                        pt = psm.tile([128, 128], F32R, tag="tp")
                        nc.tensor.transpose(
                            pt[:], R_r[:, c, k * 128:(k + 1) * 128], id_r[:])
                        rp = stgo.tile([128, 128], F32R, tag="pc")
                        nc.scalar.copy(rp[:], pt[:])
                        nc.sync.dma_start(
                            rtr_d[:, k, c * 128:(c + 1) * 128], rp[:])

                # Omega: load + cast chunk-wise -> HBM (f32r and bf16)
                for c in range(EC):
                    om_stage = stgo.tile([128, T], F32, tag="omst")
                    nc.sync.dma_start(om_stage[:], Omv[:, c, :])
                    om_rc = stgo.tile([128, T], F32R, tag="omrc")
                    nc.vector.tensor_copy(om_rc[:], om_stage[:])
                    nc.sync.dma_start(omr_d[:, c, :], om_rc[:])
                    om_bc = stgo.tile([128, T], BF16, tag="ombc")
                    nc.vector.tensor_copy(om_bc[:], om_stage[:])
                    nc.sync.dma_start(ombf_d[:, c, :], om_bc[:])
                    for tt in range(TC):
                        pt = psm.tile([128, 128], F32, tag="tp")
                        nc.tensor.transpose(
                            pt[:], om_stage[:, tt * 128:(tt + 1) * 128],
                            id_f[:])
                        otp = stgo.tile([128, 128], F32, tag="pcf")
                        nc.scalar.copy(otp[:], pt[:])
                        nc.sync.dma_start(
                            ot_d[:, tt, c * 128:(c + 1) * 128], otp[:])

                # A_scale = (R^2)^T @ Om ; inv = 1/A_scale (both -> HBM)
                with tc.tile_pool(name="stg2", bufs=2) as stg2, \
                     tc.tile_pool(name="stgq", bufs=8) as stgq:
                    for n in range(N4):
                        omsl = []
                        for c in range(EC):
                            om_t = stgq.tile([128, 512], F32R, tag="omq")
                            nc.sync.dma_start(
                                om_t[:], omr_d[:, c, n * 512:(n + 1) * 512])
                            omsl.append(om_t)
                        for fh in range(FC // 2):
                            sp = pspan.tile([128, 2, 512], F32, tag="span")
                            for j in range(2):
                                fi = fh * 2 + j
                                for c in range(EC):
                                    nc.tensor.matmul(
                                        sp[:, j, :],
                                        R2_r[:, c, fi * 128:(fi + 1) * 128],
                                        omsl[c][:],
                                        start=(c == 0), stop=(c == EC - 1))
                            asc_t = stg2.tile([128, 2, 512], F32R, tag="asc")
                            nc.scalar.copy(asc_t[:], sp[:])
                            nc.sync.dma_start(
                                asc_d[:, fh * 2:fh * 2 + 2,
                                      n * 512:(n + 1) * 512], asc_t[:])
                            invf = stg2.tile([128, 2, 512], F32, tag="invf")
                            scr = stg2.tile([128, 2, 512], F32, tag="invs")
                            nc.vector.reciprocal_approx_accurate(
                                invf[:], asc_t[:].bitcast(F32), scr[:])
                            inv_t = stg2.tile([128, 2, 512], F32R, tag="invr")
                            nc.vector.tensor_copy(inv_t[:], invf[:])
                            nc.sync.dma_start(
                                inv_d[:, fh * 2:fh * 2 + 2,
                                      n * 512:(n + 1) * 512], inv_t[:])
                            if dbg:
                                nc.sync.dma_start(
                                    dbg_d["asc"][:, fh * 2:fh * 2 + 2,
                                                 n * 512:(n + 1) * 512],
                                    asc_t[:].bitcast(F32))
                                nc.sync.dma_start(
                                    dbg_d["inv"][:, fh * 2:fh * 2 + 2,
                                                 n * 512:(n + 1) * 512],
                                    inv_t[:].bitcast(F32))

            # big per-layer residents (allocated after staging pools close)
            with tc.tile_pool(name="resB", bufs=1) as res, \
                 tc.tile_pool(name="sol", bufs=1) as sol:
                A_r = res.tile([128, FC, T], F32R)
                with tc.tile_pool(name="stgA", bufs=1) as stga:
                    for hh in range(2):
                        a_stage = stga.tile([128, FC, T // 2], F32, tag="ast")
                        sl = slice(hh * (T // 2), (hh + 1) * (T // 2))
                        nc.sync.dma_start(a_stage[:], A0v[:, :, sl])
                        nc.vector.tensor_copy(A_r[:, :, sl], a_stage[:])

                Pc = res.tile([128, EC, RK], F32)
                nc.sync.dma_start(Pc[:], P0v[:])
                Qc = res.tile([128, TC, RK], F32)
                nc.sync.dma_start(Qc[:], Q0v[:])
                err_f = res.tile([128, EC, T], F32)
                wsP = sol.tile([128, EC, RK, W_], F32)
                wsQ = sol.tile([128, TC, RK, W_], F32)

                with tc.tile_pool(name="io", bufs=2) as io, \
                     tc.tile_pool(name="io1", bufs=1) as io1, \
                     tc.tile_pool(name="wgt", bufs=18) as wgt, \
                     tc.tile_pool(name="lp", bufs=1) as lp, \
                     tc.tile_pool(name="wkc", bufs=2) as wkc, \
                     tc.tile_pool(name="cp", bufs=2) as cp, \
                     tc.tile_pool(name="cp1", bufs=1) as cp1:

                    # ============== layer loop (unrolled) ==============
                    for l in range(n_layers):
                        # --- G = R@A ; err = Om*(Y-G) ---
                        for h in range(EC // 2):
                            rts = []
                            for j in range(2):
                                m = h * 2 + j
                                for k in range(FC):
                                    rt = wgt.tile([128, 128], F32R,
                                                  tag="w_io")
                                    nc.sync.dma_start(
                                        rt[:],
                                        rtr_d[:, k, m * 128:(m + 1) * 128])
                                    rts.append(rt)
                            for n in range(N4):
                                sp = pspan.tile([128, 2, 512], F32,
                                                tag="span")
                                for j in range(2):
                                    for k in range(FC):
                                        nc.tensor.matmul(
                                            sp[:, j, :], rts[j * FC + k][:],
                                            A_r[:, k, n * 512:(n + 1) * 512],
                                            start=(k == 0),
                                            stop=(k == FC - 1))
                                y_t = io.tile([128, 2, 512], F32, tag="y_io")
                                nc.sync.dma_start(
                                    y_t[:], Yv[:, h * 2:h * 2 + 2,
                                               n * 512:(n + 1) * 512])
                                om_t = io.tile([128, 2, 512], BF16,
                                               tag="om_io")
                                nc.sync.dma_start(
                                    om_t[:], ombf_d[:, h * 2:h * 2 + 2,
                                                    n * 512:(n + 1) * 512])
                                t1 = wkc.tile([128, 2, 512], F32, tag="work")
                                nc.vector.tensor_sub(t1[:], y_t[:], sp[:])
                                nc.vector.tensor_mul(
                                    err_f[:, h * 2:h * 2 + 2,
                                          n * 512:(n + 1) * 512],
                                    om_t[:], t1[:])

                        if dbg and l == 0:
                            nc.sync.dma_start(dbg_d["err0"][:], err_f[:])

                        # --- Mp^T and rhsP^T: accumulate over T chunks ---
                        mp_ps = psm.tile([128, 512], F32, tag="acc")
                        rp_ps = psm.tile([128, 512], F32, tag="acc")
                        for tt in range(TC):
                            ot_t = io.tile([128, E], F32, tag="ot_io")
                            nc.sync.dma_start(ot_t[:], ot_d[:, tt, :])
                            qq_t = cp.tile([128, 100], F32, tag="qq")
                            nc.vector.tensor_mul(
                                qq_t[:].rearrange("p (r s) -> p r s", r=RK),
                                Qc[:, tt, :, None].to_broadcast(
                                    (128, RK, RK)),
                                Qc[:, tt, None, :].to_broadcast(
                                    (128, RK, RK)))
                            nc.tensor.matmul(mp_ps[:100, :], qq_t[:], ot_t[:],
                                             start=(tt == 0),
                                             stop=(tt == TC - 1))
                            et_t = cp.tile([128, E], F32, tag="et")
                            for c in range(EC):
                                pt = psm.tile([128, 128], F32, tag="tp")
                                nc.tensor.transpose(
                                    pt[:],
                                    err_f[:, c, tt * 128:(tt + 1) * 128],
                                    id_f[:])
                                nc.scalar.copy(
                                    et_t[:, c * 128:(c + 1) * 128], pt[:])
                            nc.tensor.matmul(rp_ps[:RK, :], Qc[:, tt, :],
                                             et_t[:], start=(tt == 0),
                                             stop=(tt == TC - 1))
                        mp_sb = cp1.tile([128, 512], F32, tag="m_sb")
                        nc.scalar.copy(mp_sb[:100, :], mp_ps[:100, :])
                        rp_sb = cp1.tile([RK, 512], F32, tag="r_sb")
                        nc.scalar.copy(rp_sb[:], rp_ps[:RK, :])
                        for c in range(EC):
                            pt = psm.tile([128, 128], F32, tag="tp")
                            nc.tensor.transpose(
                                pt[:, :100],
                                mp_sb[:100, c * 128:(c + 1) * 128],
                                id_f[:100, :100])
                            nc.scalar.copy(
                                wsP[:, c, :, :RK],
                                pt[:, :100].rearrange("p (a b) -> p a b",
                                                      a=RK))
                            pt2 = psm.tile([128, 128], F32, tag="tp")
                            nc.tensor.transpose(
                                pt2[:, :RK],
                                rp_sb[:, c * 128:(c + 1) * 128],
                                id_f[:RK, :RK])
                            nc.scalar.copy(wsP[:, c, :, RK], pt2[:, :RK])
                        dgP = wsP.rearrange("p c a b -> p c (a b)")[
                            :, :, 0:RK * W_:W_ + 1]
                        nc.vector.tensor_scalar(dgP, dgP, pcol(l, PRM_LAM),
                                                None, AL.add)

                        if dbg and l == 0:
                            nc.sync.dma_start(dbg_d["wsP"][:], wsP[:])

                        # --- P solve + update ---
                        P_hat = _gauss_solve(nc, sol, wsP, EC, "gj", "xP")
                        if dbg and l == 0:
                            nc.sync.dma_start(dbg_d["phat"][:], P_hat[:])
                        dP = sol.tile([128, EC, RK], F32, tag="dP")
                        nc.vector.tensor_sub(dP[:], P_hat[:], Pc[:])
                        nc.vector.tensor_scalar(dP[:], dP[:], pcol(l, PRM_SP),
                                                None, AL.mult)
                        nc.vector.tensor_add(Pc[:], Pc[:], dP[:])
                        Pn_r = lp.tile([128, EC, RK], F32R, tag="pn_r")
                        nc.vector.tensor_copy(Pn_r[:], Pc[:])
                        PP_f = lp.tile([128, EC, RK * RK], F32, tag="pp_f")
                        nc.vector.tensor_mul(
                            PP_f.rearrange("p c (r s) -> p c r s", r=RK)[:],
                            Pc[:, :, :, None].to_broadcast((128, EC, RK, RK)),
                            Pc[:, :, None, :].to_broadcast((128, EC, RK, RK)))

                        # --- Mq^T / rhsQ^T per 512-stripe of T ---
                        for n in range(N4):
                            mq_ps = psm.tile([128, 512], F32, tag="acc")
                            rq_ps = psm.tile([128, 512], F32, tag="acc")
                            for c in range(EC):
                                om_t = io.tile([128, 512], F32, tag="om_io2")
                                nc.sync.dma_start(
                                    om_t[:],
                                    Omv[:, c, n * 512:(n + 1) * 512])
                                nc.tensor.matmul(mq_ps[:100, :], PP_f[:, c, :],
                                                 om_t[:], start=(c == 0),
                                                 stop=(c == EC - 1))
                                nc.tensor.matmul(
                                    rq_ps[:RK, :], Pc[:, c, :],
                                    err_f[:, c, n * 512:(n + 1) * 512],
                                    start=(c == 0), stop=(c == EC - 1))
                            mq_sb = cp1.tile([128, 512], F32, tag="m_sb")
                            nc.scalar.copy(mq_sb[:100, :], mq_ps[:100, :])
                            rq_sb = cp1.tile([RK, 512], F32, tag="r_sb")
                            nc.scalar.copy(rq_sb[:], rq_ps[:RK, :])
                            for t4 in range(4):
                                tt = n * 4 + t4
                                pt = psm.tile([128, 128], F32, tag="tp")
                                nc.tensor.transpose(
                                    pt[:, :100],
                                    mq_sb[:100, t4 * 128:(t4 + 1) * 128],
                                    id_f[:100, :100])
                                nc.scalar.copy(
                                    wsQ[:, tt, :, :RK],
                                    pt[:, :100].rearrange(
                                        "p (a b) -> p a b", a=RK))
                                pt2 = psm.tile([128, 128], F32, tag="tp")
                                nc.tensor.transpose(
                                    pt2[:, :RK],
                                    rq_sb[:, t4 * 128:(t4 + 1) * 128],
                                    id_f[:RK, :RK])
                                nc.scalar.copy(wsQ[:, tt, :, RK], pt2[:, :RK])
                        dgQ = wsQ.rearrange("p c a b -> p c (a b)")[
                            :, :, 0:RK * W_:W_ + 1]
                        nc.vector.tensor_scalar(dgQ, dgQ, pcol(l, PRM_LAM),
                                                None, AL.add)

                        if dbg and l == 0:
                            nc.sync.dma_start(dbg_d["wsQ"][:], wsQ[:])

                        # --- Q solve + update ---
                        Q_hat = _gauss_solve(nc, sol, wsQ, TC, "gj", "xQ")
                        if dbg and l == 0:
                            nc.sync.dma_start(dbg_d["qhat"][:], Q_hat[:])
                        dQ = sol.tile([128, TC, RK], F32, tag="dQ")
                        nc.vector.tensor_sub(dQ[:], Q_hat[:], Qc[:])
                        nc.vector.tensor_scalar(dQ[:], dQ[:], pcol(l, PRM_SQ),
                                                None, AL.mult)
                        nc.vector.tensor_add(Qc[:], Qc[:], dQ[:])
                        Qn_r = lp.tile([128, TC, RK], F32R, tag="qn_r")
                        nc.vector.tensor_copy(Qn_r[:], Qc[:])

                        # --- P_new^T (f32r) ---
                        PnT = lp.tile([RK, E], F32R, tag="pnt")
                        for c in range(EC):
                            pt = psm.tile([128, 128], F32R, tag="tp")
                            nc.tensor.transpose(pt[:RK, :], Pn_r[:, c, :],
                                                id_r[:])
                            nc.scalar.copy(PnT[:, c * 128:(c + 1) * 128],
                                           pt[:RK, :])

                        # --- datafit stripe + C = R^T@df + A update ---
                        for n in range(N4):
                            qnt = cp1.tile([RK, 512], F32R, tag="qnt")
                            for t4 in range(4):
                                tt = n * 4 + t4
                                pt = psm.tile([128, 128], F32R, tag="tp")
                                nc.tensor.transpose(pt[:RK, :], Qn_r[:, tt, :],
                                                    id_r[:])
                                nc.scalar.copy(
                                    qnt[:, t4 * 128:(t4 + 1) * 128],
                                    pt[:RK, :])
                            df_t = cp.tile([128, EC, 512], F32R, tag="df")
                            for h in range(EC // 2):
                                sp = pspan.tile([128, 2, 512], F32,
                                                tag="span")
                                for j in range(2):
                                    m = h * 2 + j
                                    nc.tensor.matmul(
                                        sp[:, j, :],
                                        PnT[:, m * 128:(m + 1) * 128],
                                        qnt[:], start=True, stop=True)
                                om_t = io.tile([128, 2, 512], BF16,
                                               tag="om_io")
                                nc.sync.dma_start(
                                    om_t[:],
                                    ombf_d[:, h * 2:h * 2 + 2,
                                           n * 512:(n + 1) * 512])
                                q1 = wkc.tile([128, 2, 512], F32, tag="work")
                                nc.vector.tensor_mul(q1[:], om_t[:], sp[:])
                                nc.vector.tensor_sub(
                                    df_t[:, h * 2:h * 2 + 2, :],
                                    err_f[:, h * 2:h * 2 + 2,
                                          n * 512:(n + 1) * 512],
                                    q1[:])
                            if dbg and l == 0:
                                nc.sync.dma_start(
                                    dbg_d["df"][:, :,
                                                n * 512:(n + 1) * 512],
                                    df_t[:].bitcast(F32))
                            for fh in range(FC // 2):
                                rws = []
                                for j in range(2):
                                    fi = fh * 2 + j
                                    for c in range(EC):
                                        rw = wgt.tile([128, 128], F32R,
                                                      tag="w_io")
                                        nc.sync.dma_start(
                                            rw[:],
                                            rr_d[:, c,
                                                 fi * 128:(fi + 1) * 128])
                                        rws.append(rw)
                                sp = pspan.tile([128, 2, 512], F32,
                                                tag="span")
                                for j in range(2):
                                    for c in range(EC):
                                        nc.tensor.matmul(
                                            sp[:, j, :], rws[j * EC + c][:],
                                            df_t[:, c, :],
                                            start=(c == 0),
                                            stop=(c == EC - 1))
                                asc_t = io1.tile([128, 2, 512], F32R,
                                                 tag="asc_io")
                                nc.sync.dma_start(
                                    asc_t[:],
                                    asc_d[:, fh * 2:fh * 2 + 2,
                                          n * 512:(n + 1) * 512])
                                inv_t = io.tile([128, 2, 512], F32R,
                                                tag="inv_io")
                                nc.sync.dma_start(
                                    inv_t[:],
                                    inv_d[:, fh * 2:fh * 2 + 2,
                                          n * 512:(n + 1) * 512])
                                for jj in range(2):
                                    asl = A_r[:, fh * 2 + jj,
                                              n * 512:(n + 1) * 512]
                                    m1 = wkc.tile([128, 512], F32, tag="m1")
                                    nc.gpsimd.tensor_mul(
                                        m1[:], asc_t[:, jj, :], asl)
                                    d_t = wkc.tile([128, 512], F32,
                                                   tag="wrk1")
                                    if soft_resid is not None:
                                        nc.vector._custom_dve(
                                            soft_resid, out=d_t[:],
                                            in0=m1[:], in1=sp[:, jj, :],
                                            s0=pcol(l, PRM_NMU),
                                            s1=pcol(l, PRM_MU))
                                    else:
                                        nc.vector.tensor_add(
                                            m1[:], m1[:], sp[:, jj, :])
                                        nc.vector.tensor_scalar(
                                            m1[:], m1[:], pcol(l, PRM_NMU),
                                            pcol(l, PRM_MU), AL.max, AL.min)
                                        nc.vector.tensor_sub(
                                            d_t[:], sp[:, jj, :], m1[:])
                                    e_t = wkc.tile([128, 512], F32, tag="m1")
                                    nc.vector.tensor_mul(e_t[:], d_t[:],
                                                         inv_t[:, jj, :])
                                    nc.vector.affine_then_add(
                                        asl, e_t[:], asl, pcol(l, PRM_SA),
                                        0.0)

                    # ================= outputs =================
                    nc.sync.dma_start(Pov[:], Pc[:])
                    nc.sync.dma_start(Qov[:], Qc[:])
                    nc.sync.dma_start(Aov[:], A_r[:].bitcast(F32))

    nc.compile()
    return nc


_NC_CACHE = None


def _get_nc():
    global _NC_CACHE
    if _NC_CACHE is None:
        _NC_CACHE = build_bass()
    return _NC_CACHE


def make_in_maps(Y, R, Omega, P0, Q0, A0, lam_log, mu_log, skip_logits):
    lam = np.exp(lam_log.astype(np.float64)).astype(np.float32)
    mu = np.exp(mu_log.astype(np.float64)).astype(np.float32)
    sig = (1.0 / (1.0 + np.exp(-skip_logits.astype(np.float64)))).astype(
        np.float32)
    prm = np.zeros((128, NPRM), dtype=np.float32)
    for l in range(L):
        prm[:, l * 6 + PRM_LAM] = lam[l]
        prm[:, l * 6 + PRM_MU] = mu[l]
        prm[:, l * 6 + PRM_NMU] = -mu[l]
        prm[:, l * 6 + PRM_SP] = sig[l, 0]
        prm[:, l * 6 + PRM_SQ] = sig[l, 1]
        prm[:, l * 6 + PRM_SA] = sig[l, 2]
    in_maps = []
    for b in range(B):
        in_maps.append({
            "Y": np.ascontiguousarray(Y[b], dtype=np.float32),
            "R": np.ascontiguousarray(R[b], dtype=np.float32),
            "Om": np.ascontiguousarray(Omega[b], dtype=np.float32),
            "P0": np.ascontiguousarray(P0[b], dtype=np.float32),
            "Q0": np.ascontiguousarray(Q0[b], dtype=np.float32),
            "A0": np.ascontiguousarray(A0[b], dtype=np.float32),
            "prm": prm,
        })
    return in_maps


def kernel(Y, R, Omega, P0, Q0, A0, lam_log, mu_log, skip_logits):
    Y, R, Omega = np.asarray(Y), np.asarray(R), np.asarray(Omega)
    P0, Q0, A0 = np.asarray(P0), np.asarray(Q0), np.asarray(A0)
    lam_log, mu_log = np.asarray(lam_log), np.asarray(mu_log)
    skip_logits = np.asarray(skip_logits)
    nc = _get_nc()
    in_maps = make_in_maps(Y, R, Omega, P0, Q0, A0, lam_log, mu_log,
                           skip_logits)
    res = run_bass_kernel_spmd(nc, in_maps, list(range(B)))
    P = np.stack([res.results[b]["Pout"] for b in range(B)])
    Q = np.stack([res.results[b]["Qout"] for b in range(B)])
    A = np.stack([res.results[b]["Aout"] for b in range(B)])
    return (P.astype(np.float32), Q.astype(np.float32), A.astype(np.float32))
